# revision 3
# baseline (speedup 1.0000x reference)
"""Multi-head causal attention + RoPE, sharded over 8 TRN2 NeuronCores.

Sharding: core c -> batch b = c//4, head-group g = c%4 (4 of 16 heads).
Each core computes a partial output x[b] @ (its heads' slice); host sums
the 4 group partials per batch.

Device algorithm (per core, 4 heads, S=2048, D=1024, HD=64):
  - all matmul operands bf16 (keeps FWL weight loads enabled, halves
    DMA traffic and doubles DVE throughput); PSUM accumulation fp32
  - input DMAs split fine-grained and issued in consumption order
    across BOTH HWDGE queues (sync + scalar), so the first projection
    chunk lands ~5us earlier than a single-queue stream; wQ/wK are
    packed [e_k | o_k] per 128-chunk so each quarter is self-contained;
    cos/sin are packed per-sq-block; the ones-columns and rk zero
    halves are on-device memsets (no DMA dependency at all)
  - QK projections produce rope-split components via host-permuted
    weight columns; rope staged PSUM->bf16 on the early-idle ACT
    engine, rotation math on DVE in 16-bit 2x mode; scores transposed
    [sk, sq] with zero-padded per-head k tiles (K=128); exp on ACT in
    [128,1024] double-chunk tiles (bf16 out); block-causal with the
    [128,128] triangle-mask multiply on GPSIMD (SBUF-only engine,
    otherwise idle); ctx^T = [v | 1]^T @ exp gives context + softmax
    denominators in one PSUM accumulation; reciprocal via fast DVE op
  - engine load balancing across the timeline: projection/rope/v of
    block t+1 is emitted as fine-grained units interleaved into the
    attention stream of block t (the PE-paced front half), while all
    out-projections are deferred into the exp-bound later blocks where
    the PE otherwise idles; the rb/rec/ct finalize chain is deferred so
    its PE matmul never blocks the in-order queue while waiting on DVE
  - ctx matmuls trail their exp by two chunk-pairs (pexp ring of 6) so
    the PE never waits on a just-issued activation
  - PE warmed from t=0 with dummy matmuls on a memzero'd tile (no DMA
    dependency), ramping the clock before the first projection
  - output partials stored fp16 (half the write traffic); host sums in
    fp32
"""

import os
import sys

import numpy as np

for _p in ("/opt/trn_rl_repo", "/root/.axon_site/_ro/trn_rl_repo"):
    if os.path.isdir(_p) and _p not in sys.path:
        sys.path.append(_p)

import ml_dtypes  # noqa: E402

import concourse.bass as bass  # noqa: E402
import concourse.mybir as mybir  # noqa: E402
import concourse.tile as tile  # noqa: E402
from concourse import bacc  # noqa: E402
from concourse.bass import ts, ds  # noqa: E402
from concourse.bass_utils import run_bass_kernel_spmd  # noqa: E402

B, S, D = 2, 2048, 1024
HEADS, HD = 16, 64
G = 4                      # head groups == cores per batch element
HPC = HEADS // G           # heads per core
NCOL = HPC * HD            # 256 projection cols per core
KCH = D // 128             # K chunks
MCH = S // 128             # sk chunks
TCH = S // 512             # sq 512-blocks
F32 = mybir.dt.float32
F16 = mybir.dt.float16
F32R = mybir.dt.float32r
BF16 = mybir.dt.bfloat16
AF = mybir.ActivationFunctionType

# wA blob: triangle mask, then per-block cos|sin (bf16 elements)
TRI_O = 0
CS_O = TRI_O + 128         # block t: cos at CS_O+1024t, sin at +512
NA = CS_O + 2 * S

TRACE = False
TRACE_DIR = None
LAST_EXEC_NS = None
_CACHE = {}

N_WARM = 8


def _build():
    nc = bacc.Bacc("TRN2")
    xTt_d = nc.dram_tensor("xTt", (TCH, 128, KCH * 512), BF16,
                           kind="ExternalInput")
    wQ_d = nc.dram_tensor("wQ", (128, 2 * D), BF16, kind="ExternalInput")
    wK_d = nc.dram_tensor("wK", (128, 2 * D), BF16, kind="ExternalInput")
    wA_d = nc.dram_tensor("wA", (128, NA), BF16, kind="ExternalInput")
    wV_d = nc.dram_tensor("wV", (128, KCH * NCOL), BF16,
                          kind="ExternalInput")
    wO_d = nc.dram_tensor("wO", (128, 2 * D), BF16, kind="ExternalInput")
    out_d = nc.dram_tensor("out", (S, D), F16, kind="ExternalOutput")

    mm = nc.tensor.matmul

    with tile.TileContext(nc) as tc:
        with tc.tile_pool(name="pp", bufs=1) as pp, \
             tc.tile_pool(name="tmp3", bufs=3) as tmp3, \
             tc.tile_pool(name="pexp", bufs=6) as pexp, \
             tc.tile_pool(name="tmp2", bufs=2) as tmp2, \
             tc.tile_pool(name="posb", bufs=4) as posb, \
             tc.tile_pool(name="pA", bufs=1) as pA, \
             tc.tile_pool(name="psQ", bufs=2, space="PSUM") as psQ, \
             tc.tile_pool(name="psS", bufs=2, space="PSUM") as psS, \
             tc.tile_pool(name="psC", bufs=2, space="PSUM") as psC:

            # ---- PE warm-up: matmuls on zeros, no DMA dependency ----
            warm = pp.tile([128, 512], BF16)
            nc.vector.memzero(warm[:])

            def dummy_mms(nn):
                # psQ ring: no cross-engine readers, so these never pace
                # behind the exp pipeline
                for _ in range(nn):
                    dps = psQ.tile([128, 512], F32, tag="pq", name="warm")
                    mm(dps[:], warm[:, 0:128], warm[:],
                       start=True, stop=True)

            dummy_mms(4)

            # ---- input DMAs: consumption order, both HWDGE queues ----
            wQ_sb = pA.tile([128, 2 * D], BF16)
            wK_sb = pA.tile([128, 2 * D], BF16)
            wA_sb = pA.tile([128, NA], BF16)
            wV_sb = pA.tile([128, KCH * NCOL], BF16)
            wO_sb = pA.tile([128, 2 * D], BF16)
            xt = [pA.tile([128, KCH * 512], BF16, name=f"xt{t}")
                  for t in range(TCH)]

            # sync queue (free until output stores begin ~25us in)
            nc.sync.dma_start(wQ_sb[:, 0:512], wQ_d[:, 0:512])
            nc.scalar.dma_start(wQ_sb[:, 512:1024], wQ_d[:, 512:1024])
            nc.sync.dma_start(xt[0][:, 0:1024], xTt_d[0, :, 0:1024])
            nc.scalar.dma_start(xt[0][:, 1024:2048], xTt_d[0, :, 1024:2048])
            nc.sync.dma_start(wQ_sb[:, 1024:1536], wQ_d[:, 1024:1536])
            nc.scalar.dma_start(wQ_sb[:, 1536:2048], wQ_d[:, 1536:2048])
            nc.sync.dma_start(xt[0][:, 2048:3072], xTt_d[0, :, 2048:3072])
            nc.scalar.dma_start(xt[0][:, 3072:4096], xTt_d[0, :, 3072:4096])
            nc.sync.dma_start(wK_sb[:, 0:1024], wK_d[:, 0:1024])
            nc.scalar.dma_start(wK_sb[:, 1024:2048], wK_d[:, 1024:2048])
            nc.scalar.dma_start(wA_sb[:, 0:1152], wA_d[:, 0:1152])
            nc.sync.dma_start(wV_sb[:, 0:1024], wV_d[:, 0:1024])
            nc.scalar.dma_start(wA_sb[:, 1152:2176], wA_d[:, 1152:2176])
            nc.sync.dma_start(wV_sb[:, 1024:2048], wV_d[:, 1024:2048])
            nc.scalar.dma_start(wA_sb[:, 2176:4224], wA_d[:, 2176:4224])
            nc.sync.dma_start(xt[1][:, 0:2048], xTt_d[1, :, 0:2048])
            nc.sync.dma_start(xt[1][:, 2048:4096], xTt_d[1, :, 2048:4096])
            nc.sync.dma_start(xt[2][:], xTt_d[2])
            nc.sync.dma_start(xt[3][:], xTt_d[3])
            nc.sync.dma_start(wO_sb[:], wO_d[:])

            dummy_mms(N_WARM - 4)

            tri_sb = wA_sb[:, TRI_O:TRI_O + 128]
            wo_v = wO_sb[:, 0:2 * D].rearrange("p (j n) -> p j n", j=2)

            ones1 = pp.tile([1, 64], BF16)
            nc.gpsimd.memset(ones1[:], 1.0)

            # roped q per (pair, block); roped k per (head, block),
            # zero-padded to K=128 rows (other head's rows stay zero)
            rq = [[pp.tile([128, 512], BF16, name=f"rq{p}_{nb}")
                   for nb in range(TCH)] for p in range(2)]
            rk = [[pp.tile([128, 512], BF16, name=f"rk{h}_{nb}")
                   for nb in range(TCH)] for h in range(HPC)]
            # v (+ones col) per 4-chunk group so attention block t only
            # depends on groups <= t
            v_grp = [pp.tile([128, 4, HPC, HD + 1], BF16, name=f"vg{g}")
                     for g in range(TCH)]
            for g in range(TCH):
                nc.gpsimd.memset(v_grp[g][:, :, :, HD], 1.0)
            # only the 64 rows rope never writes need zeroing; block-major
            # so block 0's tiles are ready first
            for nb in range(TCH):
                for h in range(HPC):
                    r0 = 64 * (1 - h % 2)
                    nc.gpsimd.memset(rk[h][nb][r0:r0 + 64, :], 0.0)
            ct = pA.tile([128, 2, S], BF16)

            def rope_pair(e_ps, o_ps, dt_fn, nb, cpe):
                # e_ps/o_ps rows = [h0e h1e h2e h3e]/[h0o ...] x32; write
                # rotated comps of head h into dt_fn(h) (block-nb tile)
                # rows 64*(h%2)+{e:0,o:32}.  PSUM->bf16 staging runs on the
                # (early-idle) ACT engine so every DVE op gets 16-bit 2x
                # throughput
                cs = wA_sb[:, CS_O + 1024 * nb:CS_O + 1024 * nb + 512]
                sn = wA_sb[:, CS_O + 1024 * nb + 512:CS_O + 1024 * nb + 1024]
                eb = tmp3.tile([128, 512], BF16, tag="ropeb")
                ob = tmp3.tile([128, 512], BF16, tag="ropec")
                if cpe == 0:
                    nc.scalar.copy(eb[:], e_ps[:])
                    nc.scalar.copy(ob[:], o_ps[:])
                else:
                    nc.vector.tensor_copy(eb[:], e_ps[:])
                    nc.vector.tensor_copy(ob[:], o_ps[:])
                t1 = tmp3.tile([128, 512], BF16, tag="ropet")
                t2 = tmp3.tile([128, 512], BF16, tag="ropeu")
                nc.vector.tensor_mul(t1[:], eb[:], cs)
                nc.vector.tensor_mul(t2[:], ob[:], sn)
                for h in range(HPC):
                    r0 = 64 * (h % 2)
                    nc.vector.tensor_sub(
                        dt_fn(h)[r0:r0 + 32, :],
                        t1[32 * h:32 * h + 32, :],
                        t2[32 * h:32 * h + 32, :])
                t3 = tmp3.tile([128, 512], BF16, tag="ropet")
                t4 = tmp3.tile([128, 512], BF16, tag="ropeu")
                nc.vector.tensor_mul(t3[:], eb[:], sn)
                nc.vector.tensor_mul(t4[:], ob[:], cs)
                for h in range(HPC):
                    r0 = 64 * (h % 2) + 32
                    nc.vector.tensor_add(
                        dt_fn(h)[r0:r0 + 32, :],
                        t3[32 * h:32 * h + 32, :],
                        t4[32 * h:32 * h + 32, :])

            def a_units(t):
                # projection + rope + v for sq block t as a list of
                # small closures, to be interleaved into attention t-1
                units = []
                st = {}

                def qk_step(k, isq):
                    def u(k=k, isq=isq):
                        if k == 0:
                            st['e'] = psQ.tile([128, 512], F32, tag="pq",
                                               name="eps")
                            st['o'] = psQ.tile([128, 512], F32, tag="pq",
                                               name="ops")
                        we = (wQ_sb if isq else wK_sb)
                        mm(st['e'][:],
                           we[:, 256 * k:256 * k + 128],
                           xt[t][:, ts(k, 512)],
                           start=(k == 0), stop=(k == KCH - 1))
                        mm(st['o'][:],
                           we[:, 256 * k + 128:256 * k + 256],
                           xt[t][:, ts(k, 512)],
                           start=(k == 0), stop=(k == KCH - 1))
                    return u

                cpe = 1 if t == TCH - 1 else 0

                def rope_u(dt_fn):
                    def u(dt_fn=dt_fn):
                        rope_pair(st['e'], st['o'], dt_fn, t, cpe)
                    return u

                for k in range(KCH):
                    units.append(qk_step(k, True))
                units.append(rope_u(lambda h, t=t: rq[h // 2][t]))
                for k in range(KCH):
                    units.append(qk_step(k, False))
                units.append(rope_u(lambda h, t=t: rk[h][t]))

                def v_half(m4, k0):
                    def u(m4=m4, k0=k0):
                        if k0 == 0:
                            st['v'] = psQ.tile([128, NCOL], F32, tag="pq",
                                               name="vps")
                        v_ps = st['v']
                        for k in range(k0, k0 + 4):
                            mm(v_ps[:],
                               xt[t][:, 512 * k + 128 * m4:
                                     512 * k + 128 * m4 + 128],
                               wV_sb[:, NCOL * k:NCOL * (k + 1)],
                               start=(k == 0), stop=(k == KCH - 1))
                        if k0 == 4:
                            if cpe == 0:
                                nc.scalar.copy(
                                    v_grp[t][:, m4, :, 0:HD],
                                    v_ps.rearrange("p (h d) -> p h d",
                                                   h=HPC))
                            else:
                                nc.vector.tensor_copy(
                                    v_grp[t][:, m4, :, 0:HD],
                                    v_ps.rearrange("p (h d) -> p h d",
                                                   h=HPC))
                    return u

                for m4 in range(4):
                    units.append(v_half(m4, 0))
                    units.append(v_half(m4, 4))
                return units

            def outproj_units(tb, split_cp=False):
                # split_cp: alternate the PSUM->SBUF copies between DVE and
                # ACT — only for the final out-projection, when ACT is idle
                units = []
                for m4 in range(4):
                    def u(m4=m4, tb=tb):
                        m = 4 * tb + m4
                        for j2 in range(2):
                            o_ps = psS.tile([128, 512], F32, tag="sc",
                                            name="ops")
                            mm(o_ps[:], ct[:, 0, ts(m, 128)],
                               wo_v[:, 0, ts(j2, 512)],
                               start=True, stop=False)
                            mm(o_ps[:], ct[:, 1, ts(m, 128)],
                               wo_v[:, 1, ts(j2, 512)],
                               start=False, stop=True)
                            o_sb = posb.tile([128, 512], F16, tag="osb")
                            if split_cp and j2 == 1:
                                nc.scalar.copy(o_sb[:], o_ps[:])
                            else:
                                nc.vector.tensor_copy(o_sb[:], o_ps[:])
                            nc.sync.dma_start(
                                out_d[ts(m, 128), ts(j2, 512)], o_sb[:])
                    units.append(u)
                return units

            def finalize_f2(t, h, d_sb, ctx_ps_h):
                def g():
                    rb_ps = psS.tile([128, 512], F32, tag="sc", name="rb")
                    mm(rb_ps[0:64, :], ones1[:], d_sb[:],
                       start=True, stop=True)
                    rec = tmp2.tile([64, 512], F32, tag="rec")
                    nc.vector.reciprocal_approx_fast(rec[:], rb_ps[0:64, :])
                    nc.vector.tensor_mul(
                        ct[64 * (h % 2):64 * (h % 2) + 64, h // 2,
                           ts(t, 512)],
                        ctx_ps_h[0:64, :], rec[:])
                return g

            def attention_block(t, feed):
                # feed: list of closures (prior finalize chains, prior
                # out-projection, next block's projections) popped evenly
                # between score/exp emissions so the in-order PE queue
                # always has ready work; returns deferred finalize work
                nch = 4 * t + 4
                slots = 2 * (nch // 2 + 1)
                state = {'done': 0, 'slot': 0}
                prio = []

                def pop_units():
                    while prio:
                        prio.pop(0)()
                    state['slot'] += 1
                    tgt = min(len(feed),
                              (len(feed) * state['slot'] + slots - 1)
                              // slots)
                    while state['done'] < tgt:
                        feed[state['done']]()
                        state['done'] += 1

                for pair in range(2):
                    hs = (2 * pair, 2 * pair + 1)
                    ctx_ps = {h: psC.tile([65, 512], F32, tag="ctx",
                                          name=f"ctx{h}")
                              for h in hs}
                    pend = []
                    for cb in range(nch // 2):
                        c0 = 2 * cb
                        scs = {}
                        for half in range(2):
                            c = c0 + half
                            diag = (c // 4 == t)
                            off = 128 * (c % 4) if diag else 0
                            col = slice(512 * half + off, 512 * half + 512)
                            for h in hs:
                                if h not in scs:
                                    scs[h] = psS.tile(
                                        [128, 1024], F32, tag="sc",
                                        name=f"sc{h}")
                                mm(scs[h][:, col],
                                   rk[h][c // 4][:, ts(c % 4, 128)],
                                   rq[pair][t][:, ds(off, 512 - off)],
                                   start=True, stop=True)
                        nxt = []
                        # for the second diagonal chunk-pair the first 256
                        # columns are fully masked — skip them in the exp
                        lo = 256 if (c0 // 4 == t and c0 % 4 == 2) else 0
                        for h in hs:
                            sc = scs[h]
                            e_sb = pexp.tile([128, 1024], BF16, tag="exp")
                            nc.scalar.activation(e_sb[:, lo:1024],
                                                 sc[:, lo:1024], AF.Exp)
                            for half in range(2):
                                c = c0 + half
                                diag = (c // 4 == t)
                                off = 128 * (c % 4) if diag else 0
                                if diag:
                                    dcol = slice(512 * half + off,
                                                 512 * half + off + 128)
                                    nc.gpsimd.tensor_mul(
                                        e_sb[:, dcol], e_sb[:, dcol],
                                        tri_sb[:])

                                def emit_ctx(h=h, c=c, off=off,
                                             e_sb=e_sb, half=half):
                                    ecol = slice(512 * half + off,
                                                 512 * half + 512)
                                    mm(ctx_ps[h][:, ds(off, 512 - off)],
                                       v_grp[c // 4][:, c % 4, h, :],
                                       e_sb[:, ecol],
                                       start=(c == 0),
                                       stop=(c == nch - 1))
                                nxt.append(emit_ctx)
                        pop_units()
                        # ctx trails exp by two chunk-pairs so the PE
                        # never waits on a just-issued activation
                        if len(pend) == 2:
                            for f in pend.pop(0):
                                f()
                        pend.append(nxt)
                    for grp in pend:
                        for f in grp:
                            f()
                    # d_sb copies now; the rb/rec/ct chain is deferred so
                    # its PE matmul never blocks the queue while waiting
                    for h in hs:
                        d_sb = tmp2.tile([1, 512], BF16, tag="dsb")
                        nc.vector.tensor_copy(d_sb[:], ctx_ps[h][64:65, :])
                        prio.append(finalize_f2(t, h, d_sb, ctx_ps[h]))
                    if pair == 0:
                        pop_units()
                # drain remaining feed; hand deferred finalize to caller
                while state['done'] < len(feed):
                    feed[state['done']]()
                    state['done'] += 1
                return prio

            # ---- pipelined schedule ----
            # out-projections are deferred into the ACT-bound later blocks,
            # where the PE otherwise idles waiting on exp tiles
            for u in a_units(0):
                u()
            plan = [
                a_units(1),
                a_units(2),
                a_units(3) + outproj_units(0),
                outproj_units(1) + outproj_units(2),
            ]
            carry = []
            for t in range(TCH):
                feed = list(carry) + plan[t]
                carry = attention_block(t, feed)
            for u in carry:
                u()
            for u in outproj_units(TCH - 1):
                u()
    nc.compile()
    return nc


def _host_tables():
    half = HD // 2
    inv_freq = (1.0 / (10000.0 ** (np.arange(half, dtype=np.float32) / half)))
    angles = (np.arange(S, dtype=np.float32)[:, None]
              * inv_freq[None, :].astype(np.float32))
    cosT = np.tile(np.cos(angles).T.astype(np.float32), (HPC, 1))
    sinT = np.tile(np.sin(angles).T.astype(np.float32), (HPC, 1))
    i_idx = np.arange(128)[:, None]
    j_idx = np.arange(128)[None, :]
    tri = (j_idx >= i_idx).astype(np.float32)
    return cosT, sinT, tri


def _pk(w):
    # (D, M) -> partition-major (128, KCH*M)
    m = w.shape[1]
    return np.ascontiguousarray(
        w.reshape(KCH, 128, m).transpose(1, 0, 2).reshape(128, KCH * m))


def _pk_eo(we, wo):
    # (D, 128) e/o components -> (128, 2*D) interleaved [e_k | o_k]
    er = we.reshape(KCH, 128, 128)
    orr = wo.reshape(KCH, 128, 128)
    return np.ascontiguousarray(
        np.concatenate([er, orr], axis=2).transpose(1, 0, 2).reshape(
            128, 2 * D))


def kernel(x, Wq, Wk, Wv, Wo):
    global LAST_EXEC_NS
    x = np.asarray(x, dtype=np.float32)
    Wq = np.asarray(Wq, dtype=np.float32)
    Wk = np.asarray(Wk, dtype=np.float32)
    Wv = np.asarray(Wv, dtype=np.float32)
    Wo = np.asarray(Wo, dtype=np.float32)

    if "nc" not in _CACHE:
        _CACHE["nc"] = _build()
    nc = _CACHE["nc"]
    cosT, sinT, tri = _host_tables()
    bf = ml_dtypes.bfloat16

    in_maps = []
    for c in range(8):
        b, g = c // 4, c % 4
        cols = slice(g * NCOL, (g + 1) * NCOL)
        wq_g = Wq[:, cols].reshape(D, HPC, HD // 2, 2)
        wk_g = Wk[:, cols].reshape(D, HPC, HD // 2, 2)

        wQ = _pk_eo(wq_g[..., 0].reshape(D, 128) * 0.125,
                    wq_g[..., 1].reshape(D, 128) * 0.125).astype(bf)
        wK = _pk_eo(wk_g[..., 0].reshape(D, 128),
                    wk_g[..., 1].reshape(D, 128)).astype(bf)

        wA = np.zeros((128, NA), dtype=bf)
        wA[:, TRI_O:TRI_O + 128] = tri.astype(bf)
        for t in range(TCH):
            wA[:, CS_O + 1024 * t:CS_O + 1024 * t + 512] = \
                cosT[:, 512 * t:512 * (t + 1)].astype(bf)
            wA[:, CS_O + 1024 * t + 512:CS_O + 1024 * t + 1024] = \
                sinT[:, 512 * t:512 * (t + 1)].astype(bf)

        wV = _pk(Wv[:, cols]).astype(bf)
        wO = np.ascontiguousarray(
            Wo[cols, :].reshape(2, 128, D).transpose(1, 0, 2).reshape(
                128, 2 * D)).astype(bf)

        # x^T tiled: [t, p, k, m] = x[b][512t+m, 128k+p]
        xr = x[b].reshape(TCH, 512, KCH, 128)
        xTt = np.ascontiguousarray(
            xr.transpose(0, 3, 2, 1).reshape(TCH, 128, KCH * 512)).astype(bf)

        in_maps.append({
            "xTt": xTt,
            "wQ": wQ,
            "wK": wK,
            "wA": wA,
            "wV": wV,
            "wO": wO,
        })

    kw = {}
    if TRACE and TRACE_DIR:
        os.makedirs(TRACE_DIR, exist_ok=True)
        kw["tmpdir"] = TRACE_DIR
    res = run_bass_kernel_spmd(nc, in_maps, core_ids=list(range(8)),
                               trace=TRACE, **kw)
    LAST_EXEC_NS = res.exec_time_ns
    out = np.empty((B, S, D), dtype=np.float32)
    for b in range(B):
        out[b] = sum(res.results[4 * b + g]["out"].astype(np.float32)
                     for g in range(4))
    return out
